# revision 17
# baseline (speedup 1.0000x reference)
"""Channel-attention kernel for Trainium2 (Bass/Tile), self-contained.

kernel(x, beta) computes, per batch b:
    q = x[b].reshape(N, C);  E = q @ q.T;  A = softmax(E, axis=-1)
    out = beta * (A @ q) + x[b]
for x (4, 16, 16, 16, 128) fp32, beta (128,) fp32, on 8 NeuronCores.

Sharding: core k handles batch k//2, row-half k%2 (2048 of 4096 softmax rows).
Each core receives the full batch (needed for the energy/attention contraction)
plus its own row-half, and returns its (2048, 128) slice of the output.

Algorithm per core (all on-chip after two input DMAs):
  - Energy tiles are computed directly in TRANSPOSED layout (contraction index
    m on partitions, softmax row n on the free axis) so the second matmul
    (A @ q) needs no transposes of the 4096x2048 attention matrix.
  - Softmax stabilizer c[n] = ||q_n||^2 (equals the row max here since the
    diagonal dominates; any per-row constant cancels exactly in the ratio).
    It is subtracted by a rank-1 "bias matmul" accumulated into PSUM ahead of
    each energy matmul, so the exp pass on the scalar engine reads PSUM
    directly with no extra vector-engine traversal.
  - Denominator D[n] = sum_m T[m,n] via an elementwise bf16 running sum S of
    the T tiles (vector engine) folded by one ones-vector matmul.
  - Phase 2 accumulates (C, n)-layout output in PSUM over 32 m-blocks; the
    epilogue applies beta (scalar engine, per-partition scale), divides by D,
    transposes back to (n, C) on the tensor engine, and adds x.
"""

import sys

sys.path.insert(0, "/opt/trn_rl_repo")

from contextlib import ExitStack

import numpy as np

import bass_rust
import concourse.bass as bass
import concourse.tile as tile
from concourse import mybir
from concourse.masks import make_identity

F32 = mybir.dt.float32
BF16 = mybir.dt.bfloat16

B = 4             # batches
N = 4096          # rows per batch (16*16*16)
C = 128           # channels
H = 2048          # rows per core (half batch)
NB = N // 128     # 32 m-blocks
HB = H // 128     # 16 n-blocks per core
CH = 1024         # n-chunk size per pipeline pass
NCH = H // CH     # chunks per core
MM = 512          # matmul free-dim (one PSUM bank)
PE_BIAS_K = 5     # of every 8 slabs, this many use the PE rank-1 bias path
N_CORES = 8


# --- workaround: this walrus build rejects instructions carrying more than
# one sync-wait command, but Tile's scheduler can attach several waits to one
# instruction.  Post-process the scheduled program: move all but the last
# wait of each instruction onto same-engine NoOp carriers inserted directly
# before it (sequencer waits serialize, so semantics are identical). ---
_WS_COUNTER = [0]


def _split_multi_waits(nc: bass.Bass, max_waits: int = 1):
    for f in nc.m.functions:
        for bb in f.blocks:
            live = bb.instructions
            snapshot = list(live)
            off = 0
            for idx, inst in enumerate(snapshot):
                si = inst.sync_info
                if si is None:
                    continue
                waits = list(si.on_wait)
                if len(waits) <= max_waits:
                    continue
                extra, keep = waits[:-max_waits], waits[-max_waits:]
                inst.sync_info = bass_rust.SyncInfo(
                    on_wait=keep, on_update=list(si.on_update)
                )
                for w in extra:
                    _WS_COUNTER[0] += 1
                    nop = bass_rust.InstNoOp(name=f"I-waitsplit-{_WS_COUNTER[0]}")
                    nop.engine = inst.engine
                    nop.sync_info = bass_rust.SyncInfo(on_wait=[w], on_update=[])
                    live.insert(idx + off, nop)
                    off += 1


def build_kernel(nc: bass.Bass, tc: tile.TileContext, xb, xh, beta, out, reps=1):
    ctx = ExitStack()
    with ctx:
        singles = ctx.enter_context(tc.tile_pool(name="singles", bufs=1))
        t_pool = ctx.enter_context(tc.tile_pool(name="tpool", bufs=6))
        s_pool = ctx.enter_context(tc.tile_pool(name="spool", bufs=2))
        u_pool = ctx.enter_context(tc.tile_pool(name="upool", bufs=2))
        v_pool = ctx.enter_context(tc.tile_pool(name="vpool", bufs=2))
        row_pool = ctx.enter_context(tc.tile_pool(name="rowpool", bufs=2))
        e_psum = ctx.enter_context(tc.tile_pool(name="e_psum", bufs=2, space="PSUM"))
        o_psum = ctx.enter_context(tc.tile_pool(name="o_psum", bufs=1, space="PSUM"))
        d_psum = ctx.enter_context(tc.tile_pool(name="d_psum", bufs=1, space="PSUM"))
        tr_psum = ctx.enter_context(tc.tile_pool(name="tr_psum", bufs=1, space="PSUM"))

        ident_bf = singles.tile([128, 128], BF16)
        make_identity(nc, ident_bf)
        ident_f32 = singles.tile([128, 128], F32)
        make_identity(nc, ident_f32)
        ones_col = singles.tile([128, 1], BF16)
        nc.vector.memset(ones_col, 1.0)
        ones_row = singles.tile([1, 128], BF16)
        nc.vector.memset(ones_row, 1.0)
        beta_col = singles.tile([128, 1], F32)
        nc.sync.dma_start(out=beta_col, in_=beta.rearrange("(p o) -> p o", o=1))

        for _ in range(reps):
            _build_iteration(
                nc, tc, xb, xh, out,
                singles, t_pool, s_pool, u_pool, v_pool, row_pool,
                e_psum, o_psum, d_psum, tr_psum,
                ident_bf, ident_f32, ones_col, ones_row, beta_col,
            )


def _build_iteration(
    nc, tc, xb, xh, out,
    singles, t_pool, s_pool, u_pool, v_pool, row_pool,
    e_psum, o_psum, d_psum, tr_psum,
    ident_bf, ident_f32, ones_col, ones_row, beta_col,
):
    xb_r = xb.rearrange("(t p) c -> p t c", p=128)
    xh_r = xh.rearrange("(t p) c -> p t c", p=128)
    q_nat_f32 = singles.tile([128, NB, C], F32, tag="q_nat_f32")
    qh_nat_f32 = singles.tile([128, HB, C], F32, tag="qh_nat_f32")
    q_nat_bf = singles.tile([128, NB, C], BF16, tag="q_nat_bf")
    qh_nat_bf = singles.tile([128, HB, C], BF16, tag="qh_nat_bf")
    # chunked loads + casts so transposes can start early
    for g in range(NB // 8):
        gs = slice(g * 8, (g + 1) * 8)
        nc.sync.dma_start(out=q_nat_f32[:, gs, :], in_=xb_r[:, gs, :])
        nc.scalar.copy(q_nat_bf[:, gs, :], q_nat_f32[:, gs, :])
    for g in range(HB // 8):
        gs = slice(g * 8, (g + 1) * 8)
        nc.sync.dma_start(out=qh_nat_f32[:, gs, :], in_=xh_r[:, gs, :])
        nc.scalar.copy(qh_nat_bf[:, gs, :], qh_nat_f32[:, gs, :])

    # qT: q with channels on partitions, via PE transposes
    qT = singles.tile([128, N], BF16, tag="qT")
    for g in range(NB // 8):
        ptile = e_psum.tile([128, 1024], BF16, tag="e_slab")
        for j in range(8):
            t = g * 8 + j
            nc.tensor.transpose(
                ptile[:, j * 128:(j + 1) * 128], q_nat_bf[:, t, :], ident_bf
            )
        nc.scalar.copy(qT[:, g * 1024:(g + 1) * 1024], ptile)

    qTh = singles.tile([128, H], BF16, tag="qTh")
    for g in range(HB // 8):
        ptile = e_psum.tile([128, 1024], BF16, tag="e_slab")
        for j in range(8):
            t = g * 8 + j
            nc.tensor.transpose(
                ptile[:, j * 128:(j + 1) * 128], qh_nat_bf[:, t, :], ident_bf
            )
        nc.scalar.copy(qTh[:, g * 1024:(g + 1) * 1024], ptile)

    # c[n] = ||q_n||^2 in row form; negc = -c (bf16)
    qTh_sq = singles.tile([128, H], BF16, tag="qTh_sq")
    nc.vector.tensor_mul(qTh_sq, qTh, qTh)
    negc = singles.tile([1, H], BF16, tag="negc")
    for ch in range(H // MM):
        c_ps = d_psum.tile([1, MM], F32, tag="d_tile")
        nc.tensor.matmul(
            c_ps, ones_col, qTh_sq[:, ch * MM:(ch + 1) * MM], start=True, stop=True
        )
        nc.vector.tensor_scalar_mul(negc[:, ch * MM:(ch + 1) * MM], c_ps, -1.0)
    # negCb: -c[n] broadcast across all 128 partitions (via rank-1 matmul)
    negCb = singles.tile([128, H], BF16, tag="negCb")
    for ch in range(H // MM):
        b_ps = tr_psum.tile([128, MM], F32, tag="tr")
        nc.tensor.matmul(
            b_ps, ones_row, negc[:, ch * MM:(ch + 1) * MM], start=True, stop=True
        )
        nc.scalar.copy(negCb[:, ch * MM:(ch + 1) * MM], b_ps)

    for q in range(NCH):
        n0 = q * CH
        o_tile = o_psum.tile([128, CH], F32, tag="o_tile")
        # two independent denominator accumulation chains (even/odd mb)
        Ss = []
        for par in range(2):
            S_par = s_pool.tile([128, CH], BF16, tag=f"S{par}", name=f"S{par}_{q}")
            Ss.append(S_par)

        for mb in range(NB):
            pe_bias = (mb % 8) < PE_BIAS_K
            T = t_pool.tile([128, CH], BF16, tag="T")
            e_tile = e_psum.tile([128, CH], F32, tag="e_slab")
            if pe_bias:
                for ch in range(CH // MM):
                    sl = slice(ch * MM, (ch + 1) * MM)
                    nsl = slice(n0 + ch * MM, n0 + (ch + 1) * MM)
                    nc.tensor.matmul(
                        e_tile[:, sl], ones_row, negc[:, nsl],
                        start=True, stop=False,
                    )
            for ch in range(CH // MM):
                sl = slice(ch * MM, (ch + 1) * MM)
                nsl = slice(n0 + ch * MM, n0 + (ch + 1) * MM)
                nc.tensor.matmul(
                    e_tile[:, sl],
                    qT[:, mb * 128:(mb + 1) * 128],
                    qTh[:, nsl],
                    start=not pe_bias, stop=True,
                )
            if pe_bias:
                nc.scalar.activation(T, e_tile, mybir.ActivationFunctionType.Exp)
            else:
                Esub = t_pool.tile([128, CH], BF16, tag="Esub")
                nc.vector.tensor_add(Esub, e_tile, negCb[:, n0:n0 + CH])
                nc.scalar.activation(T, Esub, mybir.ActivationFunctionType.Exp)
            for ch in range(CH // MM):
                sl = slice(ch * MM, (ch + 1) * MM)
                nc.tensor.matmul(
                    o_tile[:, sl], q_nat_bf[:, mb, :], T[:, sl],
                    start=(mb == 0), stop=(mb == NB - 1),
                )
            S = Ss[mb % 2]
            if mb < 2:
                nc.vector.tensor_copy(S, T)
            else:
                nc.vector.tensor_add(S, S, T)

        # D in column form: Dcol[:, blk] = sum_par S_par_blk^T @ ones
        nblk = CH // 128
        dcol_ps = d_psum.tile([128, nblk], F32, tag="d_tile")
        for blk in range(nblk):
            for par in range(2):
                nc.tensor.matmul(
                    dcol_ps[:, blk:blk + 1],
                    Ss[par][:, blk * 128:(blk + 1) * 128],
                    ones_col, start=(par == 0), stop=(par == 1),
                )
        recipD = row_pool.tile([128, nblk], F32, tag="recipD")
        nc.vector.reciprocal(recipD, dcol_ps)

        U = u_pool.tile([128, CH], F32, tag="U")
        nc.scalar.activation(
            U, o_tile, mybir.ActivationFunctionType.Copy, scale=beta_col
        )

        V = v_pool.tile([128, CH // 128, C], F32, tag="V")
        for g in range(CH // MM):
            tr = tr_psum.tile([128, MM], F32, tag="tr")
            for j in range(MM // 128):
                blk = g * (MM // 128) + j
                nc.tensor.transpose(
                    tr[:, j * 128:(j + 1) * 128],
                    U[:, blk * 128:(blk + 1) * 128],
                    ident_f32,
                )
            for j in range(MM // 128):
                blk = g * (MM // 128) + j
                nc.vector.scalar_tensor_tensor(
                    out=V[:, blk, :],
                    in0=tr[:, j * 128:(j + 1) * 128],
                    scalar=recipD[:, blk:blk + 1],
                    in1=qh_nat_f32[:, q * (CH // 128) + blk, :],
                    op0=mybir.AluOpType.mult,
                    op1=mybir.AluOpType.add,
                )
        nc.sync.dma_start(
            out=out.rearrange("(t p) c -> p t c", p=128)[
                :, q * (CH // 128):(q + 1) * (CH // 128), :
            ],
            in_=V,
        )


def build_nc(reps: int = 1) -> bass.Bass:
    nc = bass.Bass("TRN2", target_bir_lowering=False, debug=False)
    xb = nc.dram_tensor("xb", [N, C], F32, kind="ExternalInput").ap()
    xh = nc.dram_tensor("xh", [H, C], F32, kind="ExternalInput").ap()
    beta = nc.dram_tensor("beta", [C], F32, kind="ExternalInput").ap()
    out = nc.dram_tensor("out", [H, C], F32, kind="ExternalOutput").ap()
    with tile.TileContext(nc) as tc:
        build_kernel(nc, tc, xb, xh, beta, out, reps=reps)
    _split_multi_waits(nc)
    return nc


def make_in_maps(x: np.ndarray, beta: np.ndarray):
    q = np.ascontiguousarray(x.reshape(B, N, C), dtype=np.float32)
    beta = np.ascontiguousarray(beta, dtype=np.float32)
    in_maps = []
    for k in range(N_CORES):
        b, h = k // 2, k % 2
        in_maps.append(
            {
                "xb": q[b],
                "xh": np.ascontiguousarray(q[b, h * H:(h + 1) * H]),
                "beta": beta,
            }
        )
    return in_maps


def assemble_out(results, x_shape) -> np.ndarray:
    out = np.empty((B, N, C), dtype=np.float32)
    for k in range(N_CORES):
        b, h = k // 2, k % 2
        out[b, h * H:(h + 1) * H] = results[k]["out"]
    return out.reshape(x_shape)


_CACHED_NC = None


def kernel(x: np.ndarray, beta: np.ndarray) -> np.ndarray:
    global _CACHED_NC
    from concourse import bass_utils

    if _CACHED_NC is None:
        _CACHED_NC = build_nc(reps=1)
    in_maps = make_in_maps(x, beta)
    res = bass_utils.run_bass_kernel_spmd(
        _CACHED_NC, in_maps, core_ids=list(range(N_CORES))
    )
    return assemble_out(res.results, x.shape)
